# revision 8
# baseline (speedup 1.0000x reference)
"""Chamfer loss kernel for Trainium2 (8 NeuronCores, SPMD).

Math: out = mean_i min_j d2(Xc_i, Xt_j) + mean_j min_i d2(Xc_i, Xt_j),
d2 = squared euclidean distance, clamped at 0 (clamp commutes with min).

Strategy (per core c of 8):
  - Direction 0: rows c*2048..(c+1)*2048 of Xc vs ALL of Xt -> row mins.
  - Direction 1: rows c*2048..(c+1)*2048 of Xt vs ALL of Xc -> row mins.
  Each direction is a [2048 x 16384] distance block computed on the PE via a
  K=16 matmul whose contraction rows encode d2 = x2 + y2 - 2*x.y in
  split precision: every fp32 value is split into a high part (pre-truncated
  to fp22 on the host, so the PE's fp22 input truncation is exact) and a low
  residual. Cross terms hi*hi + hi*lo + lo*hi land the product at fp32-grade
  accuracy in ONE PE pass (K-depth is free: the PE streams 1 column/cycle
  regardless of K). float32r dtype selects the single-pass fp22 path.
  Row mins are computed by chained TENSOR_TENSOR_REDUCE(min,min) ops on the
  vector engine, with the scalar engine relaying every other PSUM chunk to
  SBUF so the DVE consumes 2 distance values/cycle/lane instead of 1.
  Host side applies the clamp and the means in fp64 (exact gather work).
"""

import os
import sys

import numpy as np

_N = 16384
_NCORES = 8
_RPC = _N // _NCORES  # 2048 rows per core
_K = 16
_NTILES = _RPC // 128  # 16 row tiles per core
_GCOLS = 2048  # columns per psum tile (4 PSUM banks, 4 matmuls)
_BIG = np.float32(3.0e38)


def _chop22(x):
    """Truncate fp32 mantissa to 11 bits - matches the PE's measured fp32r
    input truncation (probe_num.py: inputs chopped to m11, products kept
    wide, accumulation fp32). Pre-truncated highs are exact on HW."""
    b = np.ascontiguousarray(np.asarray(x, np.float32)).view(np.uint32)
    return (b & np.uint32(0xFFFFF000)).view(np.float32)


def _split_points(P64):
    """P64: [n,3] fp64 points -> (Xh, Xl, sh, sl): hi/lo coordinate splits and
    hi/lo splits of the squared norms."""
    X32 = P64.astype(np.float32)
    Xh = _chop22(X32)
    Xl = (P64 - Xh.astype(np.float64)).astype(np.float32)
    s64 = (P64 * P64).sum(-1)
    sh = _chop22(s64.astype(np.float32))
    sl = (s64 - sh.astype(np.float64)).astype(np.float32)
    return Xh, Xl, sh, sl


def _lhs_matrix(Xh, Xl, sh, sl):
    """[16, n] stationary-side rows (paired with _rhs_matrix rows)."""
    n = Xh.shape[0]
    ones = np.ones(n, np.float32)
    rows = [sh, ones]
    rows += [(-2.0 * Xh[:, k]).astype(np.float32) for k in range(3)]
    rows += [sl, ones]
    rows += [(-2.0 * Xh[:, k]).astype(np.float32) for k in range(3)]
    rows += [(-2.0 * Xl[:, k]).astype(np.float32) for k in range(3)]
    rows += [(-2.0 * Xl[:, k]).astype(np.float32) for k in range(3)]
    return np.ascontiguousarray(np.stack(rows))


def _rhs_matrix(Yh, Yl, th, tl):
    """[16, n] moving-side rows."""
    n = Yh.shape[0]
    ones = np.ones(n, np.float32)
    rows = [ones, th]
    rows += [Yh[:, k] for k in range(3)]
    rows += [ones, tl]
    rows += [Yl[:, k] for k in range(3)]
    rows += [Yh[:, k] for k in range(3)]
    rows += [Yl[:, k] for k in range(3)]
    return np.ascontiguousarray(np.stack(rows))


def _emit(tc, L, R, O):
    """Emit the per-core program. L/R/O: lists of dram APs per direction."""
    from contextlib import ExitStack

    import concourse.bass as bass
    from concourse import mybir

    nc = tc.nc
    f32 = mybir.dt.float32
    f32r = mybir.dt.float32r
    AMIN = mybir.AluOpType.min

    with ExitStack() as ctx:
        rpool = ctx.enter_context(tc.tile_pool(name="rin", bufs=1))
        lpool = ctx.enter_context(tc.tile_pool(name="lin", bufs=1))
        psum = ctx.enter_context(tc.tile_pool(name="ps", bufs=2, space="PSUM"))
        accp = ctx.enter_context(tc.tile_pool(name="acc", bufs=2))
        rmp = ctx.enter_context(tc.tile_pool(name="rm", bufs=1))

        # input loads: 8 column-slices of 2048 per direction so compute can
        # start after the first slices arrive
        r_tiles = {}
        l_tiles = {}
        for d in range(2):
            l_tiles[d] = lpool.tile([_K, _RPC], f32r, tag=f"l{d}", name=f"lt{d}")
            nc.sync.dma_start(l_tiles[d][:], L[d][:])
            for g in range(_N // 2048):
                t = rpool.tile([_K, 2048], f32r, tag=f"r{d}_{g}", name=f"rt{d}_{g}")
                nc.sync.dma_start(t[:], R[d][:, g * 2048:(g + 1) * 2048])
                r_tiles[(d, g)] = t

        def emit_chunk_matmuls(d, t, c, ps, ps_off):
            """One N=512 matmul filling ps[:, ps_off:ps_off+512] with distance
            columns c*512.. for row tile t of direction d."""
            w = l_tiles[d][:, t * 128:(t + 1) * 128]
            col = c * 512
            rt = r_tiles[(d, col // 2048)]
            rhs = rt[:, col % 2048:col % 2048 + 512]
            nc.tensor.matmul(
                ps[:, ps_off:ps_off + 512], w, rhs, start=True, stop=True
            )

        rm_tiles = {}
        ngroups = _N // _GCOLS  # 2048-col groups per row
        for d in range(2):
            rm = rmp.tile([128, _NTILES], f32, tag=f"rm{d}", name=f"rmt{d}")
            rm_tiles[d] = rm
            for t in range(_NTILES):
                gm = accp.tile([128, ngroups], f32, name="gm", tag="gm")
                for g in range(ngroups):
                    ps = psum.tile([128, _GCOLS], f32, name="ps", tag="ps")
                    for m in range(_GCOLS // 512):
                        emit_chunk_matmuls(d, t, g * (_GCOLS // 512) + m, ps,
                                           m * 512)
                    nc.vector.tensor_reduce(
                        gm[:, g:g + 1], ps[:],
                        axis=mybir.AxisListType.X, op=AMIN)
                nc.vector.tensor_reduce(
                    rm[:, t:t + 1], gm[:],
                    axis=mybir.AxisListType.X, op=AMIN)
            nc.sync.dma_start(O[d][:], rm[:])


_CACHE = {}


def _build():
    if "nc" in _CACHE:
        return _CACHE["nc"]
    import concourse.bacc as bacc
    import concourse.tile as tile
    from concourse import mybir

    f32 = mybir.dt.float32
    f32r = mybir.dt.float32r
    nc = bacc.Bacc(
        "TRN2",
        target_bir_lowering=False,
        debug=False,
        num_devices=_NCORES,
    )
    L = [
        nc.dram_tensor(f"L{d}", [_K, _RPC], f32r, kind="ExternalInput").ap()
        for d in range(2)
    ]
    R = [
        nc.dram_tensor(f"R{d}", [_K, _N], f32r, kind="ExternalInput").ap()
        for d in range(2)
    ]
    O = [
        nc.dram_tensor(f"O{d}", [128, _NTILES], f32, kind="ExternalOutput").ap()
        for d in range(2)
    ]
    with tile.TileContext(nc) as tc:
        _emit(tc, L, R, O)
    nc.compile()
    _CACHE["nc"] = nc
    return nc


def make_in_maps(Xc, Xt):
    """Host-side input prep: per-core input dicts."""
    Xc64 = np.asarray(Xc, np.float64)
    Xt64 = np.asarray(Xt, np.float64)
    Xch, Xcl, sch, scl = _split_points(Xc64)
    Xth, Xtl, sth, stl = _split_points(Xt64)
    R0 = _rhs_matrix(Xth, Xtl, sth, stl)  # moving side: full Xt
    R1 = _rhs_matrix(Xch, Xcl, sch, scl)  # moving side: full Xc
    in_maps = []
    for c in range(_NCORES):
        sl = slice(c * _RPC, (c + 1) * _RPC)
        L0 = _lhs_matrix(Xch[sl], Xcl[sl], sch[sl], scl[sl])
        L1 = _lhs_matrix(Xth[sl], Xtl[sl], sth[sl], stl[sl])
        in_maps.append({"L0": L0, "R0": R0, "L1": L1, "R1": R1})
    return in_maps


def combine(results):
    """Gather per-core row mins -> final scalar (fp64 means, fp32 result)."""
    total = 0.0
    for d in range(2):
        mins = np.empty(_N, np.float32)
        for c in range(_NCORES):
            o = np.asarray(results[c][f"O{d}"])  # [128, 16]
            mins[c * _RPC:(c + 1) * _RPC] = o.T.reshape(-1)
        total += np.maximum(mins, 0).astype(np.float64).mean()
    return np.float32(total)


def kernel(Xc, Xt):
    from concourse.bass_utils import run_bass_kernel_spmd

    nc = _build()
    in_maps = make_in_maps(Xc, Xt)
    res = run_bass_kernel_spmd(nc, in_maps, list(range(_NCORES))).results
    return combine(res)


# revision 9
# speedup vs baseline: 16196.4241x; 16196.4241x over previous
"""Chamfer loss kernel for Trainium2 (8 NeuronCores, SPMD).

Math: out = mean_i min_j d2(Xc_i, Xt_j) + mean_j min_i d2(Xc_i, Xt_j),
d2 = squared euclidean distance, clamped at 0 (clamp commutes with min).

Strategy (per core c of 8):
  - Direction 0: rows c*2048..(c+1)*2048 of Xc vs ALL of Xt -> row mins.
  - Direction 1: rows c*2048..(c+1)*2048 of Xt vs ALL of Xc -> row mins.
  Each direction is a [2048 x 16384] distance block computed on the PE via a
  K=16 matmul whose contraction rows encode d2 = x2 + y2 - 2*x.y in
  split precision: every fp32 value is split into a high part (pre-truncated
  to fp22 on the host, so the PE's fp22 input truncation is exact) and a low
  residual. Cross terms hi*hi + hi*lo + lo*hi land the product at fp32-grade
  accuracy in ONE PE pass (K-depth is free: the PE streams 1 column/cycle
  regardless of K). float32r dtype selects the single-pass fp22 path.
  Row mins are computed by chained TENSOR_TENSOR_REDUCE(min,min) ops on the
  vector engine, with the scalar engine relaying every other PSUM chunk to
  SBUF so the DVE consumes 2 distance values/cycle/lane instead of 1.
  Host side applies the clamp and the means in fp64 (exact gather work).
"""

import os
import sys

import numpy as np

_N = 16384
_NCORES = 8
_RPC = _N // _NCORES  # 2048 rows per core
_K = 16
_NTILES = _RPC // 128  # 16 row tiles per core
_GCOLS = 2048  # columns per psum tile (4 PSUM banks, 4 matmuls)
_BIG = np.float32(3.0e38)


def _chop22(x):
    """Truncate fp32 mantissa to 11 bits - matches the PE's measured fp32r
    input truncation (probe_num.py: inputs chopped to m11, products kept
    wide, accumulation fp32). Pre-truncated highs are exact on HW."""
    b = np.ascontiguousarray(np.asarray(x, np.float32)).view(np.uint32)
    return (b & np.uint32(0xFFFFF000)).view(np.float32)


def _split_points(P64):
    """P64: [n,3] fp64 points -> (Xh, Xl, sh, sl): hi/lo coordinate splits and
    hi/lo splits of the squared norms."""
    X32 = P64.astype(np.float32)
    Xh = _chop22(X32)
    Xl = (P64 - Xh.astype(np.float64)).astype(np.float32)
    s64 = (P64 * P64).sum(-1)
    sh = _chop22(s64.astype(np.float32))
    sl = (s64 - sh.astype(np.float64)).astype(np.float32)
    return Xh, Xl, sh, sl


def _lhs_matrix(Xh, Xl, sh, sl):
    """[16, n] stationary-side rows (paired with _rhs_matrix rows)."""
    n = Xh.shape[0]
    ones = np.ones(n, np.float32)
    rows = [sh, ones]
    rows += [(-2.0 * Xh[:, k]).astype(np.float32) for k in range(3)]
    rows += [sl, ones]
    rows += [(-2.0 * Xh[:, k]).astype(np.float32) for k in range(3)]
    rows += [(-2.0 * Xl[:, k]).astype(np.float32) for k in range(3)]
    rows += [(-2.0 * Xl[:, k]).astype(np.float32) for k in range(3)]
    return np.ascontiguousarray(np.stack(rows))


def _rhs_matrix(Yh, Yl, th, tl):
    """[16, n] moving-side rows."""
    n = Yh.shape[0]
    ones = np.ones(n, np.float32)
    rows = [ones, th]
    rows += [Yh[:, k] for k in range(3)]
    rows += [ones, tl]
    rows += [Yl[:, k] for k in range(3)]
    rows += [Yh[:, k] for k in range(3)]
    rows += [Yl[:, k] for k in range(3)]
    return np.ascontiguousarray(np.stack(rows))


def _emit(tc, L, R, O, reps=1):
    """Emit the per-core program. L/R/O: lists of dram APs per direction."""
    from contextlib import ExitStack

    import concourse.bass as bass
    from concourse import mybir

    nc = tc.nc
    f32 = mybir.dt.float32
    f32r = mybir.dt.float32r
    AMIN = mybir.AluOpType.min

    with ExitStack() as ctx:
        rpool = ctx.enter_context(tc.tile_pool(name="rin", bufs=1))
        lpool = ctx.enter_context(tc.tile_pool(name="lin", bufs=1))
        psum = ctx.enter_context(tc.tile_pool(name="ps", bufs=2, space="PSUM"))
        accp = ctx.enter_context(tc.tile_pool(name="acc", bufs=2))
        rmp = ctx.enter_context(tc.tile_pool(name="rm", bufs=1))

        # input loads: 8 column-slices of 2048 per direction so compute can
        # start after the first slices arrive
        r_tiles = {}
        l_tiles = {}
        for d in range(2):
            l_tiles[d] = lpool.tile([_K, _RPC], f32r, tag=f"l{d}", name=f"lt{d}")
            nc.sync.dma_start(l_tiles[d][:], L[d][:])
            for g in range(_N // 2048):
                t = rpool.tile([_K, 2048], f32r, tag=f"r{d}_{g}", name=f"rt{d}_{g}")
                nc.sync.dma_start(t[:], R[d][:, g * 2048:(g + 1) * 2048])
                r_tiles[(d, g)] = t

        def emit_chunk_matmuls(d, t, c, ps, ps_off):
            """One N=512 matmul filling ps[:, ps_off:ps_off+512] with distance
            columns c*512.. for row tile t of direction d."""
            w = l_tiles[d][:, t * 128:(t + 1) * 128]
            col = c * 512
            rt = r_tiles[(d, col // 2048)]
            rhs = rt[:, col % 2048:col % 2048 + 512]
            nc.tensor.matmul(
                ps[:, ps_off:ps_off + 512], w, rhs, start=True, stop=True
            )

        rm_tiles = {}
        ngroups = _N // _GCOLS  # 2048-col groups per row
        for rep, d in [(rep, d) for rep in range(reps) for d in range(2)]:
            rm = rmp.tile([128, _NTILES], f32, tag=f"rm{d}", name=f"rmt{d}_{rep}")
            rm_tiles[d] = rm
            for t in range(_NTILES):
                gm = accp.tile([128, ngroups], f32, name="gm", tag="gm")
                for g in range(ngroups):
                    ps = psum.tile([128, _GCOLS], f32, name="ps", tag="ps")
                    for m in range(_GCOLS // 512):
                        emit_chunk_matmuls(d, t, g * (_GCOLS // 512) + m, ps,
                                           m * 512)
                    nc.vector.tensor_reduce(
                        gm[:, g:g + 1], ps[:],
                        axis=mybir.AxisListType.X, op=AMIN)
                nc.vector.tensor_reduce(
                    rm[:, t:t + 1], gm[:],
                    axis=mybir.AxisListType.X, op=AMIN)
            nc.sync.dma_start(O[d][:], rm[:])


_CACHE = {}


def _build(reps=1):
    if ("nc", reps) in _CACHE:
        return _CACHE[("nc", reps)]
    import concourse.bacc as bacc
    import concourse.tile as tile
    from concourse import mybir

    f32 = mybir.dt.float32
    f32r = mybir.dt.float32r
    nc = bacc.Bacc(
        "TRN2",
        target_bir_lowering=False,
        debug=False,
        num_devices=_NCORES,
    )
    L = [
        nc.dram_tensor(f"L{d}", [_K, _RPC], f32r, kind="ExternalInput").ap()
        for d in range(2)
    ]
    R = [
        nc.dram_tensor(f"R{d}", [_K, _N], f32r, kind="ExternalInput").ap()
        for d in range(2)
    ]
    O = [
        nc.dram_tensor(f"O{d}", [128, _NTILES], f32, kind="ExternalOutput").ap()
        for d in range(2)
    ]
    with tile.TileContext(nc) as tc:
        _emit(tc, L, R, O, reps=reps)
    nc.compile()
    _CACHE[("nc", reps)] = nc
    return nc


def make_in_maps(Xc, Xt):
    """Host-side input prep: per-core input dicts."""
    Xc64 = np.asarray(Xc, np.float64)
    Xt64 = np.asarray(Xt, np.float64)
    Xch, Xcl, sch, scl = _split_points(Xc64)
    Xth, Xtl, sth, stl = _split_points(Xt64)
    R0 = _rhs_matrix(Xth, Xtl, sth, stl)  # moving side: full Xt
    R1 = _rhs_matrix(Xch, Xcl, sch, scl)  # moving side: full Xc
    in_maps = []
    for c in range(_NCORES):
        sl = slice(c * _RPC, (c + 1) * _RPC)
        L0 = _lhs_matrix(Xch[sl], Xcl[sl], sch[sl], scl[sl])
        L1 = _lhs_matrix(Xth[sl], Xtl[sl], sth[sl], stl[sl])
        in_maps.append({"L0": L0, "R0": R0, "L1": L1, "R1": R1})
    return in_maps


def combine(results):
    """Gather per-core row mins -> final scalar (fp64 means, fp32 result)."""
    total = 0.0
    for d in range(2):
        mins = np.empty(_N, np.float32)
        for c in range(_NCORES):
            o = np.asarray(results[c][f"O{d}"])  # [128, 16]
            mins[c * _RPC:(c + 1) * _RPC] = o.T.reshape(-1)
        total += np.maximum(mins, 0).astype(np.float64).mean()
    return np.float32(total)


def kernel(Xc, Xt):
    from concourse.bass_utils import run_bass_kernel_spmd

    nc = _build()
    in_maps = make_in_maps(Xc, Xt)
    res = run_bass_kernel_spmd(nc, in_maps, list(range(_NCORES))).results
    return combine(res)
